# revision 16
# baseline (speedup 1.0000x reference)
"""Trainium2 Bass kernel for nn_DiscoveryMemory (scatter_memory).

Full computation on device across 8 NeuronCores, data-parallel over batch
(2 batches per core):
  phase 1: 1x1-conv projection (PE matmul, K=256 accumulation) with bias
           fused into the PSUM->SBUF eviction on ScalarE; pooled vector
           computed with one fused multiply+row-reduce DVE op per tile
           against a PE outer-product broadcast of preds.
  phase 2: AllGather of the 16 pooled vectors (tiny DRAM collective), then
           every core runs the sequential 16-step memory-update scan
           redundantly (branchless: one-hot/mask algebra, PE K=1
           outer-products for partition broadcasts, is_equal argmax).
  phase 3: attention. logits = memT.T @ proj; masked exp in a single
           ScalarE op (mask as per-partition bias); softmax denominator
           via an all-ones stationary matmul that lands pre-broadcast in
           PSUM; reciprocal_approx_fast; one multiply to normalize the
           aug matmul output.
"""

import sys

sys.path.insert(0, "/opt/trn_rl_repo")

import numpy as np

import concourse.bass as bass
import concourse.bacc as bacc
import concourse.mybir as mybir
import concourse.tile as tile
from concourse.bass_utils import run_bass_kernel_spmd

fp32 = mybir.dt.float32
Alu = mybir.AluOpType
Act = mybir.ActivationFunctionType

MEMSZ = 100
CODE = 128
FEATS = 256
DECAY = 0.9
N_CORES = 8
TN = 512  # free-dim tile size


def build_nc(nb, hw, n_cores, use_cc=True, stop_after='full'):
    """Build the SPMD Bass program. nb = batches per core, hw = H*W."""
    nbtot = nb * n_cores
    nt = hw // TN
    nc = bacc.Bacc("TRN2", target_bir_lowering=False, debug=False, num_devices=n_cores)

    feats_in = nc.dram_tensor("feats_sh", [nb, FEATS, hw], fp32, kind="ExternalInput")
    preds_in = nc.dram_tensor("preds_sh", [nb, hw], fp32, kind="ExternalInput")
    wt_in = nc.dram_tensor("w_projT", [FEATS, CODE], fp32, kind="ExternalInput")
    b_in = nc.dram_tensor("b_col", [CODE, 1], fp32, kind="ExternalInput")
    mem_in = nc.dram_tensor("memory0", [MEMSZ, CODE], fp32, kind="ExternalInput")
    mask_in = nc.dram_tensor("mask0", [MEMSZ, 1], fp32, kind="ExternalInput")
    oh_in = nc.dram_tensor("onehot0", [MEMSZ, 1], fp32, kind="ExternalInput")
    id100_in = nc.dram_tensor("ident100", [MEMSZ, MEMSZ], fp32, kind="ExternalInput")
    id128_in = nc.dram_tensor("ident128", [CODE, CODE], fp32, kind="ExternalInput")
    ones1x100_in = nc.dram_tensor("ones_1x100", [1, MEMSZ], fp32, kind="ExternalInput")
    ones1x128_in = nc.dram_tensor("ones_1x128", [1, CODE], fp32, kind="ExternalInput")
    onesm_in = nc.dram_tensor("ones_m", [MEMSZ, CODE], fp32, kind="ExternalInput")
    shift_in = nc.dram_tensor("shiftT", [MEMSZ, MEMSZ], fp32, kind="ExternalInput")

    out = nc.dram_tensor("out_sh", [nb, 2 * CODE, hw], fp32, kind="ExternalOutput")

    with tile.TileContext(nc) as tc:
        with (
            tc.tile_pool(name="const", bufs=1) as cpool,
            tc.tile_pool(name="proj", bufs=1) as projpool,
            tc.tile_pool(name="ft", bufs=4) as ftpool,
            tc.tile_pool(name="work", bufs=3) as wpool,
            tc.tile_pool(name="scan", bufs=2) as spool,
            tc.tile_pool(name="ps", bufs=5, space="PSUM") as pspool,
            tc.tile_pool(name="ps_small", bufs=3, space="PSUM") as psmall,
            tc.tile_pool(name="dram", bufs=1, space="DRAM") as dpool,
        ):
            # ---- constants / parameters to SBUF ----
            wt0 = cpool.tile([128, CODE], fp32)
            nc.sync.dma_start(wt0[:], wt_in[0:128, :])
            wt1 = cpool.tile([128, CODE], fp32)
            nc.sync.dma_start(wt1[:], wt_in[128:256, :])
            bcol = cpool.tile([CODE, 1], fp32)
            nc.sync.dma_start(bcol[:], b_in[:])
            id100 = cpool.tile([MEMSZ, MEMSZ], fp32)
            nc.sync.dma_start(id100[:], id100_in[:])
            id128 = cpool.tile([CODE, CODE], fp32)
            nc.sync.dma_start(id128[:], id128_in[:])
            ones1x100 = cpool.tile([1, MEMSZ], fp32)
            nc.sync.dma_start(ones1x100[:], ones1x100_in[:])
            ones1x128 = cpool.tile([1, CODE], fp32)
            nc.sync.dma_start(ones1x128[:], ones1x128_in[:])
            onesm = cpool.tile([MEMSZ, CODE], fp32)
            nc.sync.dma_start(onesm[:], onesm_in[:])
            shiftT = cpool.tile([MEMSZ, MEMSZ], fp32)
            nc.sync.dma_start(shiftT[:], shift_in[:])

            mem = spool.tile([MEMSZ, CODE], fp32, tag="mem")
            nc.sync.dma_start(mem[:], mem_in[:])
            mask = spool.tile([MEMSZ, 1], fp32, tag="mask")
            nc.sync.dma_start(mask[:], mask_in[:])
            oh = spool.tile([MEMSZ, 1], fp32, tag="oh")
            nc.sync.dma_start(oh[:], oh_in[:])

            pooled_loc = dpool.tile([nb, CODE], fp32)
            pooled_gat = dpool.tile([nbtot, CODE], fp32, addr_space="Shared")

            # ---- phase 1: projection + pooled ----
            projs = []
            for b in range(nb):
                proj_b = projpool.tile([CODE, hw], fp32, tag=f"proj{b}")
                projs.append(proj_b)
                pcols = cpool.tile([CODE, nt], fp32, tag=f"pcols{b}")

                for j in range(nt):
                    sl = slice(j * TN, (j + 1) * TN)
                    ft0 = ftpool.tile([128, TN], fp32, tag="ft0")
                    nc.sync.dma_start(ft0[:], feats_in[b, 0:128, sl])
                    ft1 = ftpool.tile([128, TN], fp32, tag="ft1")
                    nc.sync.dma_start(ft1[:], feats_in[b, 128:256, sl])
                    ps = pspool.tile([CODE, TN], fp32, tag="ps_mm")
                    nc.tensor.matmul(ps[:], wt0[:], ft0[:], start=True, stop=False)
                    nc.tensor.matmul(ps[:], wt1[:], ft1[:], start=False, stop=True)
                    # bias fused into eviction
                    nc.scalar.activation(
                        proj_b[:, sl], ps[:], Act.Identity, bias=bcol[:], scale=1.0
                    )
                    if stop_after not in ('mm',):
                        # preds broadcast [1,TN] -> [128,TN] via K=1 outer product
                        pr = ftpool.tile([1, TN], fp32, tag="pr")
                        nc.sync.dma_start(pr[:], preds_in[b : b + 1, sl])
                        pwb = pspool.tile([CODE, TN], fp32, tag="ps_mm")
                        nc.tensor.matmul(pwb[:], ones1x128[:], pr[0:1, :])
                        if stop_after not in ('pool1',):
                            junk = wpool.tile([CODE, TN], fp32, tag="junk")
                            nc.vector.scalar_tensor_tensor(
                                out=junk[:],
                                in0=proj_b[:, sl],
                                scalar=1.0,
                                in1=pwb[:],
                                op0=Alu.mult,
                                op1=Alu.mult,
                                accum_out=pcols[:, j : j + 1],
                            )
                    nc.sync.dma_start(out[b, 0:CODE, sl], proj_b[:, sl])

                if stop_after not in ('mm', 'pool1', 'pool2'):
                    pcol0 = wpool.tile([CODE, 1], fp32, tag="pcol0")
                    nc.vector.tensor_reduce(pcol0[:], pcols[:], mybir.AxisListType.X, Alu.add)
                    pcol = wpool.tile([CODE, 1], fp32, tag="pcol")
                    nc.vector.tensor_scalar(
                        out=pcol[:], in0=pcol0[:], scalar1=1.0 / hw, scalar2=None,
                        op0=Alu.mult,
                    )
                    pst = psmall.tile([1, CODE], fp32, tag="ps_s")
                    nc.tensor.transpose(pst[:], pcol[:], id128[:])
                    prow = wpool.tile([1, CODE], fp32, tag="prow")
                    nc.scalar.copy(prow[:], pst[:])
                    nc.sync.dma_start(pooled_loc[b : b + 1, :], prow[:])

            # ---- phase 2: allgather + sequential scan ----
            if n_cores > 1 and use_cc:
                nc.gpsimd.collective_compute(
                    "AllGather",
                    Alu.bypass,
                    replica_groups=[list(range(n_cores))],
                    ins=[pooled_loc.opt()],
                    outs=[pooled_gat.opt()],
                )
                gat_src = pooled_gat
            else:
                nc.sync.dma_start(pooled_gat[0:nb, :], pooled_loc[:])
                gat_src = pooled_gat

            vrow = cpool.tile([1, nbtot * CODE], fp32)
            nc.sync.dma_start(vrow[:], gat_src[:].rearrange("a b -> (a b)"))

            nsteps = 0 if stop_after in ('proj', 'mm', 'pool1', 'pool2') else nbtot
            for t in range(nsteps):
                vec = vrow[0:1, t * CODE : (t + 1) * CODE]
                # broadcast vec to all partitions
                vb = psmall.tile([MEMSZ, CODE], fp32, tag="ps_s")
                nc.tensor.matmul(vb[:], ones1x100[:], vec)
                # row norms^2 of mem (+eps to avoid inf on empty rows)
                junk_m = wpool.tile([MEMSZ, CODE], fp32, tag="junk_scan")
                n2 = wpool.tile([MEMSZ, 1], fp32, tag="n2")
                nc.vector.scalar_tensor_tensor(
                    out=junk_m[:], in0=mem[:], scalar=1.0, in1=mem[:],
                    op0=Alu.mult, op1=Alu.mult, accum_out=n2[:],
                )
                n2e = wpool.tile([MEMSZ, 1], fp32, tag="n2e")
                nc.vector.tensor_scalar(
                    out=n2e[:], in0=n2[:], scalar1=1e-20, scalar2=None, op0=Alu.add
                )
                rn2 = wpool.tile([MEMSZ, 1], fp32, tag="rn2")
                nc.vector.reciprocal(rn2[:], n2e[:])
                rn = wpool.tile([MEMSZ, 1], fp32, tag="rn")
                nc.scalar.sqrt(rn[:], rn2[:])  # 1/||mem_row||
                # dots = mem @ vec
                junk_d = wpool.tile([MEMSZ, CODE], fp32, tag="junk_scan2")
                dots = wpool.tile([MEMSZ, 1], fp32, tag="dots")
                nc.vector.scalar_tensor_tensor(
                    out=junk_d[:], in0=mem[:], scalar=1.0, in1=vb[:],
                    op0=Alu.mult, op1=Alu.mult, accum_out=dots[:],
                )
                # ||vec||-scaled sims and threshold
                junk_v = wpool.tile([1, CODE], fp32, tag="junk_v")
                vn2 = wpool.tile([1, 1], fp32, tag="vn2")
                nc.vector.scalar_tensor_tensor(
                    out=junk_v[:], in0=vec, scalar=1.0, in1=vec,
                    op0=Alu.mult, op1=Alu.mult, accum_out=vn2[:],
                )
                vn2e = wpool.tile([1, 1], fp32, tag="vn2e")
                nc.vector.tensor_scalar(
                    out=vn2e[:], in0=vn2[:], scalar1=1e-40, scalar2=None, op0=Alu.add
                )
                thresh = wpool.tile([1, 1], fp32, tag="thresh")
                nc.scalar.activation(thresh[:], vn2e[:], Act.Sqrt, scale=0.25)
                v2n = wpool.tile([1, 1], fp32, tag="v2n")
                nc.scalar.activation(v2n[:], vn2e[:], Act.Sqrt, scale=4.0)
                # sims = dots/||mem_row|| on valid slots, -2||vec|| on invalid
                offc = psmall.tile([MEMSZ, 1], fp32, tag="ps_s")
                nc.tensor.matmul(offc[:], ones1x100[:], v2n[:])  # +2||v|| all rows
                nmn = wpool.tile([MEMSZ, 1], fp32, tag="nmn")
                nc.vector.tensor_scalar(
                    out=nmn[:], in0=mask[:], scalar1=-1.0, scalar2=None, op0=Alu.add
                )  # mask-1: 0 valid, -1 invalid
                offnot = wpool.tile([MEMSZ, 1], fp32, tag="offnot")
                nc.vector.tensor_tensor(offnot[:], offc[:], nmn[:], Alu.mult)
                m1 = wpool.tile([MEMSZ, 1], fp32, tag="m1")
                nc.vector.tensor_scalar(
                    out=m1[:], in0=dots[:], scalar1=rn[:], scalar2=None, op0=Alu.mult
                )
                sims = wpool.tile([MEMSZ, 1], fp32, tag="sims")
                nc.vector.scalar_tensor_tensor(
                    out=sims[:], in0=m1[:], scalar=mask[:], in1=offnot[:],
                    op0=Alu.mult, op1=Alu.add,
                )
                # max over slots (transpose to a row first)
                simsT = psmall.tile([1, MEMSZ], fp32, tag="ps_s")
                nc.tensor.transpose(simsT[:], sims[:], id100[:])
                val = wpool.tile([1, 1], fp32, tag="val")
                nc.vector.tensor_reduce(val[:], simsT[:], mybir.AxisListType.X, Alu.max)
                flag = wpool.tile([1, 1], fp32, tag="flag")
                nc.vector.tensor_tensor(flag[:], val[:], thresh[:], Alu.is_ge)
                fb = psmall.tile([MEMSZ, 1], fp32, tag="ps_s")
                nc.tensor.matmul(fb[:], ones1x100[:], flag[:])
                valb = psmall.tile([MEMSZ, 1], fp32, tag="ps_s")
                nc.tensor.matmul(valb[:], ones1x100[:], val[:])
                heq = wpool.tile([MEMSZ, 1], fp32, tag="heq")
                nc.vector.tensor_tensor(heq[:], sims[:], valb[:], Alu.is_equal)
                h_ema = wpool.tile([MEMSZ, 1], fp32, tag="h_ema")
                nc.vector.tensor_tensor(h_ema[:], heq[:], fb[:], Alu.mult)
                nfb = wpool.tile([MEMSZ, 1], fp32, tag="nfb")
                nc.vector.tensor_scalar(
                    out=nfb[:], in0=fb[:], scalar1=-1.0, scalar2=1.0,
                    op0=Alu.mult, op1=Alu.add,
                )
                h_app = wpool.tile([MEMSZ, 1], fp32, tag="h_app")
                nc.vector.tensor_tensor(h_app[:], oh[:], nfb[:], Alu.mult)
                coefB = wpool.tile([MEMSZ, 1], fp32, tag="coefB")
                nc.vector.scalar_tensor_tensor(
                    out=coefB[:], in0=h_ema[:], scalar=1.0 - DECAY, in1=h_app[:],
                    op0=Alu.mult, op1=Alu.add,
                )
                coefA = wpool.tile([MEMSZ, 1], fp32, tag="coefA")
                nc.vector.tensor_scalar(
                    out=coefA[:], in0=coefB[:], scalar1=-1.0, scalar2=1.0,
                    op0=Alu.mult, op1=Alu.add,
                )
                tmpB = wpool.tile([MEMSZ, CODE], fp32, tag="tmpB")
                nc.vector.tensor_scalar(
                    out=tmpB[:], in0=vb[:], scalar1=coefB[:], scalar2=None, op0=Alu.mult
                )
                mem_new = spool.tile([MEMSZ, CODE], fp32, tag="mem")
                nc.vector.scalar_tensor_tensor(
                    out=mem_new[:], in0=mem[:], scalar=coefA[:], in1=tmpB[:],
                    op0=Alu.mult, op1=Alu.add,
                )
                ohs = psmall.tile([MEMSZ, 1], fp32, tag="ps_s")
                nc.tensor.matmul(ohs[:], shiftT[:], oh[:])
                t_oh = wpool.tile([MEMSZ, 1], fp32, tag="t_oh")
                nc.vector.tensor_tensor(t_oh[:], ohs[:], nfb[:], Alu.mult)
                oh_new = spool.tile([MEMSZ, 1], fp32, tag="oh")
                nc.vector.scalar_tensor_tensor(
                    out=oh_new[:], in0=oh[:], scalar=fb[:], in1=t_oh[:],
                    op0=Alu.mult, op1=Alu.add,
                )
                mask_new = spool.tile([MEMSZ, 1], fp32, tag="mask")
                nc.vector.tensor_tensor(mask_new[:], mask[:], h_app[:], Alu.add)
                mem, oh, mask = mem_new, oh_new, mask_new

            # ---- phase 2.5: memT + mask bias ----
            mtps = psmall.tile([CODE, MEMSZ], fp32, tag="ps_s")
            nc.tensor.transpose(mtps[:], mem[:], id100[:])
            memT = cpool.tile([CODE, MEMSZ], fp32)
            nc.scalar.copy(memT[:], mtps[:])
            maskbias = cpool.tile([MEMSZ, 1], fp32)
            nc.vector.tensor_scalar(
                out=maskbias[:], in0=mask[:], scalar1=1e30, scalar2=-1e30,
                op0=Alu.mult, op1=Alu.add,
            )

            # ---- phase 3: attention ----
            nbat = 0 if stop_after in ('proj', 'scan', 'mm', 'pool1', 'pool2') else nb
            for b in range(nbat):
                proj_b = projs[b]
                for j in range(nt):
                    sl = slice(j * TN, (j + 1) * TN)
                    lg = pspool.tile([MEMSZ, TN], fp32, tag="ps_mm")
                    nc.tensor.matmul(lg[:], memT[:], proj_b[:, sl])
                    e = wpool.tile([MEMSZ, TN], fp32, tag="e")
                    nc.scalar.activation(
                        e[:], lg[:], Act.Exp, bias=maskbias[:], scale=1.0
                    )
                    den = pspool.tile([CODE, TN], fp32, tag="ps_mm")
                    nc.tensor.matmul(den[:], onesm[:], e[:])
                    aug = pspool.tile([CODE, TN], fp32, tag="ps_mm")
                    nc.tensor.matmul(aug[:], mem[:], e[:])
                    r = wpool.tile([CODE, TN], fp32, tag="r")
                    nc.vector.reciprocal_approx_fast(r[:], den[:])
                    outa = wpool.tile([CODE, TN], fp32, tag="outa")
                    nc.vector.tensor_tensor(outa[:], aug[:], r[:], Alu.mult)
                    nc.sync.dma_start(out[b, CODE : 2 * CODE, sl], outa[:])

    nc.compile()
    return nc


_CACHE = {}


def _get_nc(nb, hw, n_cores):
    key = (nb, hw, n_cores)
    if key not in _CACHE:
        _CACHE[key] = build_nc(nb, hw, n_cores)
    return _CACHE[key]


def make_in_maps(feats, preds, w_proj, b_proj, memory, ptr, n_cores=N_CORES):
    B, F, H, W = feats.shape
    hw = H * W
    nb = B // n_cores
    ptr = int(ptr)
    consts = {
        "w_projT": np.ascontiguousarray(w_proj.T).astype(np.float32),
        "b_col": np.ascontiguousarray(b_proj.reshape(CODE, 1)).astype(np.float32),
        "memory0": np.ascontiguousarray(memory).astype(np.float32),
        "mask0": (np.arange(MEMSZ) < ptr).astype(np.float32).reshape(MEMSZ, 1),
        "onehot0": (np.arange(MEMSZ) == ptr).astype(np.float32).reshape(MEMSZ, 1),
        "ident100": np.eye(MEMSZ, dtype=np.float32),
        "ident128": np.eye(CODE, dtype=np.float32),
        "ones_1x100": np.ones((1, MEMSZ), np.float32),
        "ones_1x128": np.ones((1, CODE), np.float32),
        "ones_m": np.ones((MEMSZ, CODE), np.float32),
        "shiftT": np.eye(MEMSZ, k=1, dtype=np.float32),
    }
    in_maps = []
    for i in range(n_cores):
        sh = {
            "feats_sh": np.ascontiguousarray(
                feats[i * nb : (i + 1) * nb].reshape(nb, F, hw)
            ).astype(np.float32),
            "preds_sh": np.ascontiguousarray(
                preds[i * nb : (i + 1) * nb].reshape(nb, hw)
            ).astype(np.float32),
        }
        sh.update(consts)
        in_maps.append(sh)
    return in_maps


def assemble_output(results, B, H, W, n_cores=N_CORES):
    nb = B // n_cores
    parts = [results[i]["out_sh"].reshape(nb, 2 * CODE, H, W) for i in range(n_cores)]
    return np.concatenate(parts, axis=0)


def kernel(feats, preds, w_proj, b_proj, memory, ptr):
    B, F, H, W = feats.shape
    hw = H * W
    nb = B // N_CORES
    nc = _get_nc(nb, hw, N_CORES)
    in_maps = make_in_maps(feats, preds, w_proj, b_proj, memory, ptr, N_CORES)
    res = run_bass_kernel_spmd(nc, in_maps, core_ids=list(range(N_CORES)))
    return assemble_output(res.results, B, H, W, N_CORES)
